# revision 1
# baseline (speedup 1.0000x reference)
"""LoRA MultiheadAttention on 8 NeuronCores (Bass/Tile).

Sharding: 32 (batch, head) attention slices -> 4 heads x 1 batch per core.
Cores 0-3 take batch 0, cores 4-7 batch 1; core c handles heads
(c%4)*4 .. (c%4)*4+3, i.e. a contiguous 256-wide slice of the head dims.

Per-core math (all big matmuls bf16 on PE, fp32 PSUM accumulate):
  xaT   (1152, 2048) = [X^T; ones-row; zero pad]  (bias via ones row)
  qkT   = wqk^T-slices @ X  -> Q^T, K^T in (head-dim, T) layout
          (q pre-scaled by 1/sqrt(hd); LoRA K accumulated into same PSUM)
  V     = X @ Wv-slice (natural (T, dv) layout, per-head 65-wide blocks with
          a ones column -> PV matmul emits the softmax denominator for free)
  S^T   = K^T.T-slices @ Q^T  (tj on partitions, ti free)  [K=64 contraction]
  P^T   = exp(S^T)  on ACT  (no max-subtraction: |scores| <~ 3 by construction)
  O^T   = V_aug.T @ P^T  accumulated over tj; row 64 = denom
  norm  : denom row broadcast across 64 partitions via K=1 PE matmul with a
          ones column, reciprocal on DVE, multiply -> normalized O^T (bf16)
  out   = O^T.T @ out_w-slice^T  (T, 1024) fp32 partial, summed on host.

b_v is folded into the V matmul ones-row bias; out_b added on host.
"""

import sys

sys.path.insert(0, "/opt/trn_rl_repo")

import math
from contextlib import ExitStack

import ml_dtypes
import numpy as np

import concourse.bass as bass
import concourse.tile as tile
from concourse import bacc
from concourse import mybir
from concourse.bass_utils import run_bass_kernel_spmd

BF16 = ml_dtypes.bfloat16
F32 = mybir.dt.float32
BF = mybir.dt.bfloat16

T = 2048
D = 1024
H = 16
HD = 64
R = 16
BSZ = 2
SCALE = 16.0
NCORES = 8
HPC = 4  # heads per core
CD = HPC * HD  # 256 head dims per core
VW = HD + 1  # V block width per head (ones column appended)
KPAD = 1152  # 1024 X rows + 1 ones row, padded to 9 k-tiles of 128
NKT = KPAD // 128
P = 128
NTT = T // P  # 16 row tiles
HF = T // 2  # 1024: ti processed in two halves


def build_nc():
    nc = bass.Bass()
    xa = nc.dram_tensor("xa", [KPAD, T], BF, kind="ExternalInput")
    wqk = nc.dram_tensor("wqk", [KPAD, 2 * CD], BF, kind="ExternalInput")
    wv = nc.dram_tensor("wv", [KPAD, HPC * VW], BF, kind="ExternalInput")
    ab = nc.dram_tensor("ab", [KPAD, 3 * R], BF, kind="ExternalInput")
    kbm = nc.dram_tensor("kbm", [R, CD], BF, kind="ExternalInput")
    vbm = nc.dram_tensor("vbm", [R, HPC * VW], BF, kind="ExternalInput")
    wo = nc.dram_tensor("wo", [CD, D], BF, kind="ExternalInput")
    out = nc.dram_tensor("out", [T, D], F32, kind="ExternalOutput")

    with tile.TileContext(nc) as tc, ExitStack() as ctx:
        singles = ctx.enter_context(tc.tile_pool(name="singles", bufs=1))

        xa_t = [singles.tile([P, T], BF, name=f"xa{i}", tag=f"xa{i}") for i in range(NKT)]
        wqk_t = [singles.tile([P, 2 * CD], BF, name=f"wqk{i}", tag=f"wqk{i}") for i in range(NKT)]
        wv_t = [singles.tile([P, HPC * VW], BF, name=f"wv{i}", tag=f"wv{i}") for i in range(NKT)]
        ab_t = [singles.tile([P, 3 * R], BF, name=f"ab{i}", tag=f"ab{i}") for i in range(NKT)]
        kb_t = singles.tile([R, CD], BF, tag="kb")
        vb_t = singles.tile([R, HPC * VW], BF, tag="vb")
        wo_t = [singles.tile([P, D], BF, name=f"wo{i}", tag=f"wo{i}") for i in range(2)]
        for i in range(NKT):
            nc.sync.dma_start(out=xa_t[i], in_=xa[i * P : (i + 1) * P, :])
            nc.sync.dma_start(out=wqk_t[i], in_=wqk[i * P : (i + 1) * P, :])
            nc.sync.dma_start(out=wv_t[i], in_=wv[i * P : (i + 1) * P, :])
            nc.sync.dma_start(out=ab_t[i], in_=ab[i * P : (i + 1) * P, :])
        nc.sync.dma_start(out=kb_t, in_=kbm[:, :])
        nc.sync.dma_start(out=vb_t, in_=vbm[:, :])
        for i in range(2):
            nc.sync.dma_start(out=wo_t[i], in_=wo[i * P : (i + 1) * P, :])

        ones_t = singles.tile([1, HD], F32, tag="ones")
        nc.vector.memset(ones_t, 1.0)

        qk_sb = [singles.tile([P, T], BF, name=f"qk{i}", tag=f"qk{i}") for i in range(4)]
        ak_sb = singles.tile([R, T], BF, tag="ak")
        av_sb = singles.tile([R, T], BF, tag="av")
        v_sb = [singles.tile([P, HPC * VW], BF, name=f"v{i}", tag=f"v{i}") for i in range(NTT)]
        oT_sb = [singles.tile([P, T], BF, name=f"oT{i}", tag=f"oT{i}") for i in range(2)]

        # Phase A: A_kv^T = [k_a; v_a] @ X   (32, T)
        with tc.tile_pool(name="pA", bufs=2, space="PSUM") as pA:
            for ch in range(4):
                cs = slice(ch * 512, (ch + 1) * 512)
                pa = pA.tile([3 * R, 512], F32, tag="pa")
                for kt in range(8):  # ab rows >= 1024 are zero; skip 9th tile
                    nc.tensor.matmul(
                        pa,
                        lhsT=ab_t[kt],
                        rhs=xa_t[kt][:, cs],
                        start=(kt == 0),
                        stop=(kt == 7),
                    )
                nc.vector.tensor_copy(ak_sb[:, cs], pa[0:R, :])
                nc.vector.tensor_copy(av_sb[:, cs], pa[2 * R : 3 * R, :])

        # Phase B: Q^T, K^T (4 m-tiles of 128) with LoRA-K accumulated
        with tc.tile_pool(name="pB", bufs=3, space="PSUM") as pB:
            for m in range(4):
                for ch in range(4):
                    cs = slice(ch * 512, (ch + 1) * 512)
                    pq = pB.tile([P, 512], F32, tag="pq")
                    for kt in range(NKT):
                        nc.tensor.matmul(
                            pq,
                            lhsT=wqk_t[kt][:, m * P : (m + 1) * P],
                            rhs=xa_t[kt][:, cs],
                            start=(kt == 0),
                            stop=(kt == NKT - 1 and m < 2),
                        )
                    if m >= 2:
                        nc.tensor.matmul(
                            pq,
                            lhsT=kb_t[:, (m - 2) * P : (m - 1) * P],
                            rhs=ak_sb[:, cs],
                            start=False,
                            stop=True,
                        )
                    nc.vector.tensor_copy(qk_sb[m][:, cs], pq)

        # Phase C: V natural (T, 4*65) with ones cols + b_v via ones-row, LoRA-V
        with tc.tile_pool(name="pC", bufs=3, space="PSUM") as pC:
            for mt in range(NTT):
                ms = slice(mt * P, (mt + 1) * P)
                pv = pC.tile([P, HPC * VW], F32, tag="pv")
                for kt in range(NKT):
                    nc.tensor.matmul(
                        pv,
                        lhsT=xa_t[kt][:, ms],
                        rhs=wv_t[kt],
                        start=(kt == 0),
                        stop=False,
                    )
                nc.tensor.matmul(
                    pv, lhsT=av_sb[:, ms], rhs=vb_t, start=False, stop=True
                )
                nc.vector.tensor_copy(v_sb[mt], pv)

        # Phase D+E: attention units (half-outer, head-inner), software-
        # pipelined normalize (unit i's normalize emitted after unit i+1's
        # matmuls so PE never stalls at unit boundaries), denominator
        # broadcast via DRAM round-trip DMA (stride-0 partition read) instead
        # of a PE matmul, and half-0 out-proj overlapped with half-1 attention.
        with (
            tc.tile_pool(name="pS", bufs=3, space="PSUM") as pS,
            tc.tile_pool(name="pO", bufs=2, space="PSUM") as pO,
            tc.tile_pool(name="pE", bufs=1, space="PSUM") as pE,
            tc.tile_pool(name="pP", bufs=6) as pP,
            tc.tile_pool(name="pN", bufs=2) as pN,
            tc.tile_pool(name="pD", bufs=2, space="DRAM") as pD,
            tc.tile_pool(name="pOut", bufs=3) as pOut,
        ):
            def emit_unit(half, h):
                qT = qk_sb[h // 2][(h % 2) * HD : (h % 2) * HD + HD, :]
                kT = qk_sb[2 + h // 2][(h % 2) * HD : (h % 2) * HD + HD, :]
                po = pO.tile([VW, HF], F32, tag="po", name=f"po_{half}_{h}")
                pts = {}

                def emit_pv(tjp):
                    for q2 in range(2):
                        nc.tensor.matmul(
                            po[:, q2 * 512 : (q2 + 1) * 512],
                            lhsT=v_sb[tjp][:, h * VW : (h + 1) * VW],
                            rhs=pts.pop((tjp, q2)),
                            start=(tjp == 0),
                            stop=(tjp == NTT - 1),
                        )

                # PV shifted one tj behind S so exp(tj) overlaps S(tj+1) and
                # PE never waits on ACT (keeps the >=3us continuous-execution
                # window that promotes PE to the full 2.4 GHz p-state).
                for tj in range(NTT):
                    for q2 in range(2):
                        qs = slice(half * HF + q2 * 512, half * HF + (q2 + 1) * 512)
                        ps = pS.tile([P, 512], F32, tag="spsum", name=f"ps_{half}_{h}_{tj}_{q2}")
                        nc.tensor.matmul(
                            ps,
                            lhsT=kT[:, tj * P : (tj + 1) * P],
                            rhs=qT[:, qs],
                            start=True,
                            stop=True,
                        )
                        pt = pP.tile([P, 512], BF, tag="pt", name=f"pt_{half}_{h}_{tj}_{q2}")
                        nc.scalar.activation(pt, ps, mybir.ActivationFunctionType.Exp)
                        pts[(tj, q2)] = pt
                    if tj > 0:
                        emit_pv(tj - 1)
                emit_pv(NTT - 1)
                return po

            def emit_norm(half, h, po):
                hs = slice(half * HF, (half + 1) * HF)
                den = pN.tile([1, HF], F32, tag="den", name=f"den_{half}_{h}")
                nc.vector.tensor_copy(den, po[HD:VW, :])
                dr = pD.tile([1, HF], F32, tag="dr", name=f"dr_{half}_{h}")
                nc.sync.dma_start(out=dr, in_=den)
                den64 = pN.tile([HD, HF], F32, tag="den64", name=f"den64_{half}_{h}")
                nc.sync.dma_start(
                    out=den64,
                    in_=bass.AP(tensor=dr.tensor, offset=dr.offset, ap=[[0, HD], [1, HF]]),
                )
                rec = pN.tile([HD, HF], F32, tag="rec", name=f"rec_{half}_{h}")
                nc.vector.reciprocal(rec, den64)
                nc.vector.tensor_mul(
                    oT_sb[h // 2][(h % 2) * HD : (h % 2) * HD + HD, hs],
                    po[0:HD, :],
                    rec,
                )

            def emit_outproj(half):
                for mt in range(half * 8, (half + 1) * 8):
                    ms = slice(mt * P, (mt + 1) * P)
                    ob = pOut.tile([P, D], F32, tag="ob", name=f"ob_{mt}")
                    for ch in range(2):
                        cs = slice(ch * 512, (ch + 1) * 512)
                        po2 = pE.tile([P, 512], F32, tag="po2", name=f"po2_{mt}_{ch}")
                        for kt2 in range(2):
                            nc.tensor.matmul(
                                po2,
                                lhsT=oT_sb[kt2][:, ms],
                                rhs=wo_t[kt2][:, cs],
                                start=(kt2 == 0),
                                stop=(kt2 == 1),
                            )
                        nc.vector.tensor_copy(ob[:, cs], po2)
                    nc.sync.dma_start(out=out[ms, :], in_=ob)

            units = [(half, h) for half in range(2) for h in range(HPC)]
            prev = None
            for i, (half, h) in enumerate(units):
                po = emit_unit(half, h)
                if prev is not None:
                    emit_norm(prev[0], prev[1], prev[2])
                    if i == 4:
                        emit_outproj(0)
                prev = (half, h, po)
            emit_norm(prev[0], prev[1], prev[2])
            emit_outproj(1)

    # bass.Bass's finalize skips Bacc's wait-splitting passes; walrus allows
    # at most 1 sync wait per instruction (2 for event semaphores), so run
    # just those two passes here.
    import bass_rust as _bass_rust

    _bass_rust.move_matmul_waits_to_ldweights(nc.m)
    _bass_rust.generate_event_semaphores(nc)
    return nc


def prepare_in_maps(inputs):
    q = np.asarray(inputs["query"], np.float32)
    ipw = np.asarray(inputs["in_proj_weight"], np.float32)
    ipb = np.asarray(inputs["in_proj_bias"], np.float32)
    out_w = np.asarray(inputs["out_w"], np.float32)
    k_a = np.asarray(inputs["k_a"], np.float32)
    k_b = np.asarray(inputs["k_b"], np.float32)
    v_a = np.asarray(inputs["v_a"], np.float32)
    v_b = np.asarray(inputs["v_b"], np.float32)
    qscale = 1.0 / math.sqrt(HD)
    sl = SCALE / R

    in_maps = []
    for c in range(NCORES):
        bb = c // 4
        s = (c % 4) * CD
        e = s + CD
        X = q[:, bb, :]

        xa = np.zeros((KPAD, T), np.float32)
        xa[:D] = X.T
        xa[D] = 1.0

        wqk = np.zeros((KPAD, 2 * CD), np.float32)
        wqk[:D, :CD] = ipw[s:e].T * qscale
        wqk[D, :CD] = ipb[s:e] * qscale
        wqk[:D, CD:] = ipw[D + s : D + e].T
        wqk[D, CD:] = ipb[D + s : D + e]

        wv = np.zeros((KPAD, HPC * VW), np.float32)
        for j in range(HPC):
            wv[:D, j * VW : j * VW + HD] = ipw[2 * D + s + j * HD : 2 * D + s + (j + 1) * HD].T
            wv[D, j * VW : j * VW + HD] = ipb[2 * D + s + j * HD : 2 * D + s + (j + 1) * HD]
            wv[D, j * VW + HD] = 1.0

        ab = np.zeros((KPAD, 3 * R), np.float32)
        ab[:D, :R] = k_a.T
        ab[:D, 2 * R :] = v_a.T

        kbm = k_b[:, s:e] * sl

        vbm = np.zeros((R, HPC * VW), np.float32)
        for j in range(HPC):
            vbm[:, j * VW : j * VW + HD] = v_b[:, s + j * HD : s + (j + 1) * HD] * sl

        wo = out_w[:, s:e].T

        in_maps.append(
            {
                "xa": xa.astype(BF16),
                "wqk": wqk.astype(BF16),
                "wv": wv.astype(BF16),
                "ab": ab.astype(BF16),
                "kbm": kbm.astype(BF16),
                "vbm": vbm.astype(BF16),
                "wo": wo.astype(BF16),
            }
        )
    return in_maps


def assemble_output(inputs, results):
    out_b = np.asarray(inputs["out_b"], np.float32)
    out = np.zeros((T, BSZ, D), np.float32)
    for c in range(NCORES):
        out[:, c // 4, :] += results[c]["out"]
    out += out_b[None, None, :]
    return out


def kernel(**inputs):
    nc = build_nc()
    in_maps = prepare_in_maps(inputs)
    res = run_bass_kernel_spmd(nc, in_maps, core_ids=list(range(NCORES)))
    return assemble_output(inputs, res.results)



# revision 9
# speedup vs baseline: 1.5129x; 1.5129x over previous
"""LoRA MultiheadAttention on 8 NeuronCores (Bass/Tile).

Sharding: 32 (batch, head) attention slices -> 4 heads x 1 batch per core.
Cores 0-3 take batch 0, cores 4-7 batch 1; core c handles heads
(c%4)*4 .. (c%4)*4+3, i.e. a contiguous 256-wide slice of the head dims.

Per-core math (all big matmuls bf16 on PE, fp32 PSUM accumulate):
  xaT   (1152, 2048) = [X^T; ones-row; zero pad]  (bias via ones row)
  qkT   = wqk^T-slices @ X  -> Q^T, K^T in (head-dim, T) layout
          (q pre-scaled by 1/sqrt(hd); LoRA K accumulated into same PSUM)
  V     = X @ Wv-slice (natural (T, dv) layout, per-head 65-wide blocks with
          a ones column -> PV matmul emits the softmax denominator for free)
  S^T   = K^T.T-slices @ Q^T  (tj on partitions, ti free)  [K=64 contraction]
  P^T   = exp(S^T)  on ACT  (no max-subtraction: |scores| <~ 3 by construction)
  O^T   = V_aug.T @ P^T  accumulated over tj; row 64 = denom
  norm  : denom row broadcast across 64 partitions via K=1 PE matmul with a
          ones column, reciprocal on DVE, multiply -> normalized O^T (bf16)
  out   = O^T.T @ out_w-slice^T  (T, 1024) fp32 partial, summed on host.

b_v is folded into the V matmul ones-row bias; out_b added on host.
"""

import sys

sys.path.insert(0, "/opt/trn_rl_repo")

import math
from contextlib import ExitStack

import ml_dtypes
import numpy as np

import concourse.bass as bass
import concourse.tile as tile
from concourse import bacc
from concourse import mybir
from concourse.bass_utils import run_bass_kernel_spmd

BF16 = ml_dtypes.bfloat16
F32 = mybir.dt.float32
BF = mybir.dt.bfloat16

T = 2048
D = 1024
H = 16
HD = 64
R = 16
BSZ = 2
SCALE = 16.0
NCORES = 8
HPC = 4  # heads per core
CD = HPC * HD  # 256 head dims per core
VW = HD + 1  # V block width per head (ones column appended)
KPAD = 1152  # 1024 X rows + 1 ones row, padded to 9 k-tiles of 128
NKT = KPAD // 128
P = 128
NTT = T // P  # 16 row tiles
HF = T // 2  # 1024: ti processed in two halves


def build_nc():
    nc = bass.Bass()
    xa = nc.dram_tensor("xa", [KPAD, T], BF, kind="ExternalInput")
    wqk = nc.dram_tensor("wqk", [KPAD, 2 * CD], BF, kind="ExternalInput")
    wv = nc.dram_tensor("wv", [KPAD, HPC * VW], BF, kind="ExternalInput")
    ab = nc.dram_tensor("ab", [KPAD, 3 * R], BF, kind="ExternalInput")
    kbm = nc.dram_tensor("kbm", [R, CD], BF, kind="ExternalInput")
    vbm = nc.dram_tensor("vbm", [R, HPC * VW], BF, kind="ExternalInput")
    wo = nc.dram_tensor("wo", [CD, D], BF, kind="ExternalInput")
    out = nc.dram_tensor("out", [T, D], BF, kind="ExternalOutput")

    with tile.TileContext(nc) as tc, ExitStack() as ctx:
        singles = ctx.enter_context(tc.tile_pool(name="singles", bufs=1))

        xa_t = [singles.tile([P, T], BF, name=f"xa{i}", tag=f"xa{i}") for i in range(NKT)]
        wqk_t = [singles.tile([P, 2 * CD], BF, name=f"wqk{i}", tag=f"wqk{i}") for i in range(NKT)]
        wv_t = [singles.tile([P, HPC * VW], BF, name=f"wv{i}", tag=f"wv{i}") for i in range(NKT)]
        ab_t = [singles.tile([P, 3 * R], BF, name=f"ab{i}", tag=f"ab{i}") for i in range(NKT)]
        kb_t = singles.tile([R, CD], BF, tag="kb")
        vb_t = singles.tile([R, HPC * VW], BF, tag="vb")
        wo_t = [singles.tile([P, D], BF, name=f"wo{i}", tag=f"wo{i}") for i in range(2)]
        for i in range(NKT):
            nc.sync.dma_start(out=xa_t[i], in_=xa[i * P : (i + 1) * P, :])
            nc.sync.dma_start(out=wqk_t[i], in_=wqk[i * P : (i + 1) * P, :])
            nc.sync.dma_start(out=wv_t[i], in_=wv[i * P : (i + 1) * P, :])
            nc.sync.dma_start(out=ab_t[i], in_=ab[i * P : (i + 1) * P, :])
        nc.sync.dma_start(out=kb_t, in_=kbm[:, :])
        nc.sync.dma_start(out=vb_t, in_=vbm[:, :])
        for i in range(2):
            nc.sync.dma_start(out=wo_t[i], in_=wo[i * P : (i + 1) * P, :])

        ones_t = singles.tile([1, HD], F32, tag="ones")
        nc.vector.memset(ones_t, 1.0)

        # Dummy exp with no deps: walrus's ACT_TABLE_LOAD for the exp set
        # (~2.7us) runs during the input DMA wait instead of at the first
        # real attention exp.
        scr_t = singles.tile([1, HD], BF, tag="scr")
        nc.scalar.activation(scr_t, ones_t, mybir.ActivationFunctionType.Exp)

        qk_sb = [singles.tile([P, T], BF, name=f"qk{i}", tag=f"qk{i}") for i in range(4)]
        ak_sb = singles.tile([R, T], BF, tag="ak")
        av_sb = singles.tile([R, T], BF, tag="av")
        v_sb = [singles.tile([P, HPC * VW], BF, name=f"v{i}", tag=f"v{i}") for i in range(NTT)]
        oT_sb = [singles.tile([P, T], BF, name=f"oT{i}", tag=f"oT{i}") for i in range(2)]

        # Phase A: A_kv^T = [k_a; v_a] @ X   (32, T)
        with tc.tile_pool(name="pA", bufs=2, space="PSUM") as pA:
            for ch in range(4):
                cs = slice(ch * 512, (ch + 1) * 512)
                pa = pA.tile([3 * R, 512], F32, tag="pa")
                for kt in range(8):  # ab rows >= 1024 are zero; skip 9th tile
                    nc.tensor.matmul(
                        pa,
                        lhsT=ab_t[kt],
                        rhs=xa_t[kt][:, cs],
                        start=(kt == 0),
                        stop=(kt == 7),
                    )
                nc.vector.tensor_copy(ak_sb[:, cs], pa[0:R, :])
                nc.vector.tensor_copy(av_sb[:, cs], pa[2 * R : 3 * R, :])

        # Phase B: Q^T, K^T (4 m-tiles of 128) with LoRA-K accumulated
        with tc.tile_pool(name="pB", bufs=3, space="PSUM") as pB:
            for m in range(4):
                for ch in range(4):
                    cs = slice(ch * 512, (ch + 1) * 512)
                    pq = pB.tile([P, 512], F32, tag="pq")
                    for kt in range(NKT):
                        nc.tensor.matmul(
                            pq,
                            lhsT=wqk_t[kt][:, m * P : (m + 1) * P],
                            rhs=xa_t[kt][:, cs],
                            start=(kt == 0),
                            stop=(kt == NKT - 1 and m < 2),
                        )
                    if m >= 2:
                        nc.tensor.matmul(
                            pq,
                            lhsT=kb_t[:, (m - 2) * P : (m - 1) * P],
                            rhs=ak_sb[:, cs],
                            start=False,
                            stop=True,
                        )
                    nc.vector.tensor_copy(qk_sb[m][:, cs], pq)

        # Phase C: V natural (T, 4*65) with ones cols + b_v via ones-row, LoRA-V
        with tc.tile_pool(name="pC", bufs=3, space="PSUM") as pC:
            for mt in range(NTT):
                ms = slice(mt * P, (mt + 1) * P)
                pv = pC.tile([P, HPC * VW], F32, tag="pv")
                for kt in range(NKT):
                    nc.tensor.matmul(
                        pv,
                        lhsT=xa_t[kt][:, ms],
                        rhs=wv_t[kt],
                        start=(kt == 0),
                        stop=False,
                    )
                nc.tensor.matmul(
                    pv, lhsT=av_sb[:, ms], rhs=vb_t, start=False, stop=True
                )
                nc.vector.tensor_copy(v_sb[mt], pv)

        # Phase D+E: attention units (half-outer, head-inner). The attention
        # phase is ACT-exp-bound (16.8M exps/core = 109us floor at 1 elem/
        # lane/cycle @1.2GHz), so everything is arranged to keep ACT 100%
        # busy on 1024-wide exp tiles ((1024+352)/1.2 ~= 1.15us each, vs
        # 2x720ns for two 512-wide ones) and to keep the PE's stall gaps
        # short enough (<3.4us) that HAM keeps it at the 2.4 GHz p-state.
        # PSUM budget (8 banks): pS 2x(128,1024)=4, pO 1x(65,1024)=2,
        # pX 2x(128,512)=2. Normalize copies po out of PSUM first so the
        # single po bank pair frees early, uses the DRAM round-trip DMA for
        # the denominator broadcast, and reciprocal_approx_fast (~5x faster
        # than InstReciprocal; denominators are ~1e2..1e4 so the approx's
        # edge cases are unreachable and 18 bits is plenty).
        with (
            tc.tile_pool(name="pS", bufs=2, space="PSUM") as pS,
            tc.tile_pool(name="pO", bufs=1, space="PSUM") as pO,
            tc.tile_pool(name="pX", bufs=2, space="PSUM") as pX,
            tc.tile_pool(name="pP", bufs=3) as pP,
            tc.tile_pool(name="pN", bufs=2) as pN,
            tc.tile_pool(name="pD", bufs=2, space="DRAM") as pD,
            tc.tile_pool(name="pOut", bufs=3) as pOut,
        ):
            def emit_unit(half, h):
                qT = qk_sb[h // 2][(h % 2) * HD : (h % 2) * HD + HD, :]
                kT = qk_sb[2 + h // 2][(h % 2) * HD : (h % 2) * HD + HD, :]
                po = pO.tile([VW, HF], F32, tag="po", name=f"po_{half}_{h}")
                pts = {}

                def emit_pv(tjp):
                    pt = pts.pop(tjp)
                    for q2 in range(2):
                        nc.tensor.matmul(
                            po[:, q2 * 512 : (q2 + 1) * 512],
                            lhsT=v_sb[tjp][:, h * VW : (h + 1) * VW],
                            rhs=pt[:, q2 * 512 : (q2 + 1) * 512],
                            start=(tjp == 0),
                            stop=(tjp == NTT - 1),
                        )

                # PV shifted one tj behind S so exp(tj) overlaps S(tj+1).
                for tj in range(NTT):
                    ps = pS.tile([P, HF], F32, tag="spsum", name=f"ps_{half}_{h}_{tj}")
                    for q2 in range(2):
                        qs = slice(half * HF + q2 * 512, half * HF + (q2 + 1) * 512)
                        nc.tensor.matmul(
                            ps[:, q2 * 512 : (q2 + 1) * 512],
                            lhsT=kT[:, tj * P : (tj + 1) * P],
                            rhs=qT[:, qs],
                            start=True,
                            stop=True,
                        )
                    pt = pP.tile([P, HF], BF, tag="pt", name=f"pt_{half}_{h}_{tj}")
                    nc.scalar.activation(pt, ps, mybir.ActivationFunctionType.Exp)
                    pts[tj] = pt
                    if tj > 0:
                        emit_pv(tj - 1)
                emit_pv(NTT - 1)
                return po

            def emit_norm(half, h, po):
                hs = slice(half * HF, (half + 1) * HF)
                un = pN.tile([VW, HF], F32, tag="un", name=f"un_{half}_{h}")
                nc.vector.tensor_copy(un, po)  # frees the single po bank pair
                dr = pD.tile([1, HF], F32, tag="dr", name=f"dr_{half}_{h}")
                nc.sync.dma_start(out=dr, in_=un[HD:VW, :])
                den64 = pN.tile([HD, HF], F32, tag="den64", name=f"den64_{half}_{h}")
                nc.sync.dma_start(
                    out=den64,
                    in_=bass.AP(tensor=dr.tensor, offset=dr.offset, ap=[[0, HD], [1, HF]]),
                )
                rec = pN.tile([HD, HF], F32, tag="rec", name=f"rec_{half}_{h}")
                nc.vector.reciprocal(rec, den64)
                nc.vector.tensor_mul(
                    oT_sb[h // 2][(h % 2) * HD : (h % 2) * HD + HD, hs],
                    un[0:HD, :],
                    rec,
                )

            def emit_outproj(mts, act_copies):
                # act_copies: at the tail ACT is idle, so give it half the
                # PSUM->SBUF cast copies; during attention ACT is the
                # bottleneck so keep all copies on DVE.
                for mt in mts:
                    ms = slice(mt * P, (mt + 1) * P)
                    ob = pOut.tile([P, D], BF, tag="ob", name=f"ob_{mt}")
                    for ch in range(2):
                        cs = slice(ch * 512, (ch + 1) * 512)
                        px = pX.tile([P, 512], F32, tag="px", name=f"px_{mt}_{ch}")
                        for kt2 in range(2):
                            nc.tensor.matmul(
                                px,
                                lhsT=oT_sb[kt2][:, ms],
                                rhs=wo_t[kt2][:, cs],
                                start=(kt2 == 0),
                                stop=(kt2 == 1),
                            )
                        if act_copies and ch == 1:
                            nc.scalar.copy(ob[:, cs], px)
                        else:
                            nc.vector.tensor_copy(ob[:, cs], px)
                    nc.sync.dma_start(out=out[ms, :], in_=ob)

            units = [(half, h) for half in range(2) for h in range(HPC)]
            prev = None
            for i, (half, h) in enumerate(units):
                po = emit_unit(half, h)
                if prev is not None:
                    emit_norm(prev[0], prev[1], prev[2])
                    if i >= 4:
                        # half-0 out-proj, 2 row-tiles per unit so the PE
                        # insert never starves ACT of its next exp tile
                        emit_outproj([2 * (i - 4), 2 * (i - 4) + 1], act_copies=False)
                prev = (half, h, po)
            emit_norm(prev[0], prev[1], prev[2])
            emit_outproj(list(range(8, 16)), act_copies=True)

    # bass.Bass's finalize skips Bacc's wait-splitting passes; walrus allows
    # at most 1 sync wait per instruction (2 for event semaphores), so run
    # just those two passes here.
    import bass_rust as _bass_rust

    _bass_rust.move_matmul_waits_to_ldweights(nc.m)
    _bass_rust.generate_event_semaphores(nc)
    return nc


def prepare_in_maps(inputs):
    q = np.asarray(inputs["query"], np.float32)
    ipw = np.asarray(inputs["in_proj_weight"], np.float32)
    ipb = np.asarray(inputs["in_proj_bias"], np.float32)
    out_w = np.asarray(inputs["out_w"], np.float32)
    k_a = np.asarray(inputs["k_a"], np.float32)
    k_b = np.asarray(inputs["k_b"], np.float32)
    v_a = np.asarray(inputs["v_a"], np.float32)
    v_b = np.asarray(inputs["v_b"], np.float32)
    qscale = 1.0 / math.sqrt(HD)
    sl = SCALE / R

    in_maps = []
    for c in range(NCORES):
        bb = c // 4
        s = (c % 4) * CD
        e = s + CD
        X = q[:, bb, :]

        xa = np.zeros((KPAD, T), np.float32)
        xa[:D] = X.T
        xa[D] = 1.0

        wqk = np.zeros((KPAD, 2 * CD), np.float32)
        wqk[:D, :CD] = ipw[s:e].T * qscale
        wqk[D, :CD] = ipb[s:e] * qscale
        wqk[:D, CD:] = ipw[D + s : D + e].T
        wqk[D, CD:] = ipb[D + s : D + e]

        wv = np.zeros((KPAD, HPC * VW), np.float32)
        for j in range(HPC):
            wv[:D, j * VW : j * VW + HD] = ipw[2 * D + s + j * HD : 2 * D + s + (j + 1) * HD].T
            wv[D, j * VW : j * VW + HD] = ipb[2 * D + s + j * HD : 2 * D + s + (j + 1) * HD]
            wv[D, j * VW + HD] = 1.0

        ab = np.zeros((KPAD, 3 * R), np.float32)
        ab[:D, :R] = k_a.T
        ab[:D, 2 * R :] = v_a.T

        kbm = k_b[:, s:e] * sl

        vbm = np.zeros((R, HPC * VW), np.float32)
        for j in range(HPC):
            vbm[:, j * VW : j * VW + HD] = v_b[:, s + j * HD : s + (j + 1) * HD] * sl

        wo = out_w[:, s:e].T

        in_maps.append(
            {
                "xa": xa.astype(BF16),
                "wqk": wqk.astype(BF16),
                "wv": wv.astype(BF16),
                "ab": ab.astype(BF16),
                "kbm": kbm.astype(BF16),
                "vbm": vbm.astype(BF16),
                "wo": wo.astype(BF16),
            }
        )
    return in_maps


def assemble_output(inputs, results):
    out_b = np.asarray(inputs["out_b"], np.float32)
    out = np.zeros((T, BSZ, D), np.float32)
    for c in range(NCORES):
        out[:, c // 4, :] += results[c]["out"].astype(np.float32)
    out += out_b[None, None, :]
    return out


def kernel(**inputs):
    nc = build_nc()
    in_maps = prepare_in_maps(inputs)
    res = run_bass_kernel_spmd(nc, in_maps, core_ids=list(range(NCORES)))
    return assemble_output(inputs, res.results)



# revision 11
# speedup vs baseline: 1.7387x; 1.1492x over previous
"""LoRA MultiheadAttention on 8 NeuronCores (Bass/Tile).

Sharding: 32 (batch, head) attention slices -> 4 heads x 1 batch per core.
Cores 0-3 take batch 0, cores 4-7 batch 1; core c handles heads
(c%4)*4 .. (c%4)*4+3, i.e. a contiguous 256-wide slice of the head dims.

Per-core math (all big matmuls bf16 on PE, fp32 PSUM accumulate):
  qkT   = wqk^T-slices @ X  -> Q^T, K^T in (head-dim, T) layout
          (q pre-scaled by 1/sqrt(hd)); Q bias added as a per-partition
          tensor_scalar during the PSUM->SBUF copy, K bias folded into a
          17th row of the LoRA-K accumulation matmul (ones row in A^T)
  V     = X @ Wv-slice, per-head 65-wide blocks with a ones column ->
          the PV matmul emits the softmax denominator for free; V bias +
          the ones-column constant folded into a 17th LoRA-V row
  S^T   = K^T.T-slices @ Q^T  (tj on partitions, ti free)  [K=64]
  P^T   = exp(S^T) on ACT, 1024-wide tiles (no max-subtraction: |s|<~3)
  O^T   = V_aug.T @ P^T accumulated over tj; row 64 = denom
  norm  : denom -> DRAM -> (64,16) reshape -> cheap reciprocal ->
          DRAM -> (64,1024) stride-0 broadcast -> multiply
  out   = O^T.T @ out_w-slice^T, bf16 partials summed on host.

The attention phase is ACT-exp-bound in isolation (16.8M exps/core =
109us floor at 1 elem/lane/cycle @1.2GHz) but total PE work slightly
exceeds it, so Phase B(m1,m3) + all of Phase C + half the out-proj are
woven into the attention units' PE stream as fillers: ACT runs
continuously while the PE never gaps >3.4us (keeps HAM at 2.4 GHz).

out_b added on host.
"""

import sys

sys.path.insert(0, "/opt/trn_rl_repo")

import math
from contextlib import ExitStack

import ml_dtypes
import numpy as np

import concourse.bass as bass
import concourse.tile as tile
from concourse import bacc
from concourse import mybir
from concourse.bass_utils import run_bass_kernel_spmd

BF16 = ml_dtypes.bfloat16
F32 = mybir.dt.float32
BF = mybir.dt.bfloat16

T = 2048
D = 1024
H = 16
HD = 64
R = 16
RA = R + 1  # LoRA rank + ones row (bias folding)
BSZ = 2
SCALE = 16.0
NCORES = 8
HPC = 4  # heads per core
CD = HPC * HD  # 256 head dims per core
VW = HD + 1  # V block width per head (ones column appended)
NKT = D // 128  # 8 contraction k-tiles (no bias row: biases are folded)
P = 128
NTT = T // P  # 16 row tiles
HF = T // 2  # 1024: ti processed in two halves


def build_nc():
    nc = bass.Bass()
    xa = nc.dram_tensor("xa", [D, T], BF, kind="ExternalInput")
    wqk = nc.dram_tensor("wqk", [D, 2 * CD], BF, kind="ExternalInput")
    wv = nc.dram_tensor("wv", [D, HPC * VW], BF, kind="ExternalInput")
    ab = nc.dram_tensor("ab", [D, 3 * R], BF, kind="ExternalInput")
    kbm = nc.dram_tensor("kbm", [RA, CD], BF, kind="ExternalInput")
    vbm = nc.dram_tensor("vbm", [RA, HPC * VW], BF, kind="ExternalInput")
    qb = nc.dram_tensor("qb", [P, 2], F32, kind="ExternalInput")
    wo = nc.dram_tensor("wo", [CD, D], BF, kind="ExternalInput")
    out = nc.dram_tensor("out", [T, D], BF, kind="ExternalOutput")

    with tile.TileContext(nc) as tc, ExitStack() as ctx:
        singles = ctx.enter_context(tc.tile_pool(name="singles", bufs=1))

        xa_t = [singles.tile([P, T], BF, name=f"xa{i}", tag=f"xa{i}") for i in range(NKT)]
        wqk_t = [singles.tile([P, 2 * CD], BF, name=f"wqk{i}", tag=f"wqk{i}") for i in range(NKT)]
        wv_t = [singles.tile([P, HPC * VW], BF, name=f"wv{i}", tag=f"wv{i}") for i in range(NKT)]
        ab_t = [singles.tile([P, 3 * R], BF, name=f"ab{i}", tag=f"ab{i}") for i in range(NKT)]
        kb_t = singles.tile([RA, CD], BF, tag="kb")
        vb_t = singles.tile([RA, HPC * VW], BF, tag="vb")
        qb_t = singles.tile([P, 2], F32, tag="qb")
        wo_t = [singles.tile([P, D], BF, name=f"wo{i}", tag=f"wo{i}") for i in range(2)]
        # DMA issue order matches consumption: A needs ab+xa, then B needs
        # wqk, C (interleaved into unit 0) needs wv, out-proj needs wo last.
        for i in range(NKT):
            nc.sync.dma_start(out=ab_t[i], in_=ab[i * P : (i + 1) * P, :])
            nc.sync.dma_start(out=xa_t[i], in_=xa[i * P : (i + 1) * P, :])
        for i in range(NKT):
            nc.sync.dma_start(out=wqk_t[i], in_=wqk[i * P : (i + 1) * P, :])
        for i in range(NKT):
            nc.sync.dma_start(out=wv_t[i], in_=wv[i * P : (i + 1) * P, :])
        nc.sync.dma_start(out=kb_t, in_=kbm[:, :])
        nc.sync.dma_start(out=vb_t, in_=vbm[:, :])
        nc.sync.dma_start(out=qb_t, in_=qb[:, :])
        for i in range(2):
            nc.sync.dma_start(out=wo_t[i], in_=wo[i * P : (i + 1) * P, :])

        ones_t = singles.tile([1, HD], F32, tag="ones")
        nc.vector.memset(ones_t, 1.0)

        # Dummy exp with no deps: walrus's ACT_TABLE_LOAD for the exp set
        # (~2.7us) runs during the input DMA wait instead of at the first
        # real attention exp.
        scr_t = singles.tile([1, HD], BF, tag="scr")
        nc.scalar.activation(scr_t, ones_t, mybir.ActivationFunctionType.Exp)

        qk_sb = [singles.tile([P, T], BF, name=f"qk{i}", tag=f"qk{i}") for i in range(4)]
        ak_sb = singles.tile([RA, T], BF, tag="ak")
        av_sb = singles.tile([RA, T], BF, tag="av")
        v_sb = [singles.tile([P, HPC * VW], BF, name=f"v{i}", tag=f"v{i}") for i in range(NTT)]
        oT_sb = [singles.tile([P, T], BF, name=f"oT{i}", tag=f"oT{i}") for i in range(2)]

        # ones row for the bias-folding contraction (row 16 of A^T tiles):
        # engines can't address a 1-partition region at base 16, so memset
        # the whole tile and let Phase A overwrite rows 0-15.
        nc.vector.memset(ak_sb, 1.0)
        nc.vector.memset(av_sb, 1.0)

        # Phase A: A_kv^T = [k_a; v_a] @ X   (rows 0..15 of ak/av)
        with tc.tile_pool(name="pA", bufs=2, space="PSUM") as pA:
            for ch in range(4):
                cs = slice(ch * 512, (ch + 1) * 512)
                pa = pA.tile([3 * R, 512], F32, tag="pa")
                for kt in range(NKT):
                    nc.tensor.matmul(
                        pa,
                        lhsT=ab_t[kt],
                        rhs=xa_t[kt][:, cs],
                        start=(kt == 0),
                        stop=(kt == NKT - 1),
                    )
                nc.vector.tensor_copy(ak_sb[0:R, cs], pa[0:R, :])
                nc.vector.tensor_copy(av_sb[0:R, cs], pa[2 * R : 3 * R, :])

        # Phase B prologue: Q^T heads 0-1 (m=0) and K^T heads 0-1 (m=2).
        # m=1 and m=3 are emitted as attention fillers below.
        def emit_b(pool, m):
            for ch in range(4):
                cs = slice(ch * 512, (ch + 1) * 512)
                pq = pool.tile([P, 512], F32, tag=pool._bias_tag, name=f"pq_{m}_{ch}")
                for kt in range(NKT):
                    nc.tensor.matmul(
                        pq,
                        lhsT=wqk_t[kt][:, m * P : (m + 1) * P],
                        rhs=xa_t[kt][:, cs],
                        start=(kt == 0),
                        stop=(kt == NKT - 1 and m < 2),
                    )
                if m >= 2:
                    # LoRA-K + K bias (ones row 16 x kbm bias row)
                    nc.tensor.matmul(
                        pq,
                        lhsT=kb_t[:, (m - 2) * P : (m - 1) * P],
                        rhs=ak_sb[:, cs],
                        start=False,
                        stop=True,
                    )
                    nc.vector.tensor_copy(qk_sb[m][:, cs], pq)
                else:
                    # Q bias as a per-partition scalar during the copy
                    nc.vector.tensor_scalar_add(qk_sb[m][:, cs], pq, qb_t[:, m : m + 1])

        with tc.tile_pool(name="pB", bufs=3, space="PSUM") as pB:
            pB._bias_tag = "pq"
            emit_b(pB, 0)
            emit_b(pB, 2)

        # Phase D+E: attention units (half-outer, head-inner) with Phase
        # B(m1,m3), Phase C, and half-0 out-proj woven in as PE fillers.
        # PSUM budget (8 banks): pS 2x(128,1024)=4, pO 1x(65,1024)=2,
        # pX 2x(128,512)=2 (shared by B/C fillers and out-proj).
        with (
            tc.tile_pool(name="pS", bufs=2, space="PSUM") as pS,
            tc.tile_pool(name="pO", bufs=1, space="PSUM") as pO,
            tc.tile_pool(name="pX", bufs=2, space="PSUM") as pX,
            tc.tile_pool(name="pP", bufs=3) as pP,
            tc.tile_pool(name="pN", bufs=2) as pN,
            tc.tile_pool(name="pD", bufs=2, space="DRAM") as pD,
            tc.tile_pool(name="pOut", bufs=3) as pOut,
        ):
            pX._bias_tag = "px"

            def filler_c(mt):
                def f():
                    ms = slice(mt * P, (mt + 1) * P)
                    pv = pX.tile([P, 512], F32, tag="px", name=f"pv_{mt}")
                    for kt in range(NKT):
                        nc.tensor.matmul(
                            pv[:, : HPC * VW],
                            lhsT=xa_t[kt][:, ms],
                            rhs=wv_t[kt],
                            start=(kt == 0),
                            stop=False,
                        )
                    # LoRA-V + V bias + ones-column constant (row 16)
                    nc.tensor.matmul(
                        pv[:, : HPC * VW], lhsT=av_sb[:, ms], rhs=vb_t,
                        start=False, stop=True,
                    )
                    nc.vector.tensor_copy(v_sb[mt], pv[:, : HPC * VW])
                return f

            def filler_b(m, ch):
                def f():
                    cs = slice(ch * 512, (ch + 1) * 512)
                    pq = pX.tile([P, 512], F32, tag="px", name=f"pq_{m}_{ch}")
                    for kt in range(NKT):
                        nc.tensor.matmul(
                            pq,
                            lhsT=wqk_t[kt][:, m * P : (m + 1) * P],
                            rhs=xa_t[kt][:, cs],
                            start=(kt == 0),
                            stop=(kt == NKT - 1 and m < 2),
                        )
                    if m >= 2:
                        nc.tensor.matmul(
                            pq,
                            lhsT=kb_t[:, (m - 2) * P : (m - 1) * P],
                            rhs=ak_sb[:, cs],
                            start=False,
                            stop=True,
                        )
                        nc.vector.tensor_copy(qk_sb[m][:, cs], pq)
                    else:
                        nc.vector.tensor_scalar_add(
                            qk_sb[m][:, cs], pq, qb_t[:, m : m + 1]
                        )
                return f

            def emit_outproj(mts, act_copies):
                for mt in mts:
                    ms = slice(mt * P, (mt + 1) * P)
                    ob = pOut.tile([P, D], BF, tag="ob", name=f"ob_{mt}")
                    for ch in range(2):
                        cs = slice(ch * 512, (ch + 1) * 512)
                        px = pX.tile([P, 512], F32, tag="px", name=f"px_{mt}_{ch}")
                        for kt2 in range(2):
                            nc.tensor.matmul(
                                px,
                                lhsT=oT_sb[kt2][:, ms],
                                rhs=wo_t[kt2][:, cs],
                                start=(kt2 == 0),
                                stop=(kt2 == 1),
                            )
                        if act_copies and ch == 1:
                            nc.scalar.copy(ob[:, cs], px)
                        else:
                            nc.vector.tensor_copy(ob[:, cs], px)
                    nc.sync.dma_start(out=out[ms, :], in_=ob)

            def emit_unit(half, h, fillers):
                qT = qk_sb[h // 2][(h % 2) * HD : (h % 2) * HD + HD, :]
                kT = qk_sb[2 + h // 2][(h % 2) * HD : (h % 2) * HD + HD, :]
                po = pO.tile([VW, HF], F32, tag="po", name=f"po_{half}_{h}")
                pts = {}

                def emit_pv(tjp):
                    pt = pts.pop(tjp)
                    for q2 in range(2):
                        nc.tensor.matmul(
                            po[:, q2 * 512 : (q2 + 1) * 512],
                            lhsT=v_sb[tjp][:, h * VW : (h + 1) * VW],
                            rhs=pt[:, q2 * 512 : (q2 + 1) * 512],
                            start=(tjp == 0),
                            stop=(tjp == NTT - 1),
                        )

                # PV shifted one tj behind S so exp(tj) overlaps S(tj+1);
                # one filler block per tj slot keeps the ACT queue fed while
                # sneaking prologue/out-proj PE work into ACT-idle slack.
                for tj in range(NTT):
                    ps = pS.tile([P, HF], F32, tag="spsum", name=f"ps_{half}_{h}_{tj}")
                    for q2 in range(2):
                        qs = slice(half * HF + q2 * 512, half * HF + (q2 + 1) * 512)
                        nc.tensor.matmul(
                            ps[:, q2 * 512 : (q2 + 1) * 512],
                            lhsT=kT[:, tj * P : (tj + 1) * P],
                            rhs=qT[:, qs],
                            start=True,
                            stop=True,
                        )
                    pt = pP.tile([P, HF], BF, tag="pt", name=f"pt_{half}_{h}_{tj}")
                    nc.scalar.activation(pt, ps, mybir.ActivationFunctionType.Exp)
                    pts[tj] = pt
                    if fillers:
                        fillers.pop(0)()
                    if tj > 0:
                        emit_pv(tj - 1)
                emit_pv(NTT - 1)
                return po

            # Normalize, split so the single po bank pair is evacuated
            # immediately after the unit's last PV (emit_norm_copy) while
            # the division happens later (emit_norm_finish), keeping the
            # next unit's PV from stalling on it.  The reciprocal runs on
            # a (64,16) DMA-reshape of the denominator row (DVE's
            # InstReciprocal is ~7.4 cyc/elem, so the (64,1024) broadcast
            # form costs 7.8us; reshaped it is ~0.2us), then a second DMA
            # round trip broadcasts the reciprocal across 64 partitions.
            def emit_norm_copy(half, h, po):
                un = pN.tile([VW, HF], F32, tag="un", name=f"un_{half}_{h}")
                nc.vector.tensor_copy(un, po)
                dr = pD.tile([1, HF], F32, tag="dr", name=f"dr_{half}_{h}")
                nc.sync.dma_start(out=dr, in_=un[HD:VW, :])
                rs = pN.tile([HD, 16], F32, tag="rs", name=f"rs_{half}_{h}")
                nc.sync.dma_start(
                    out=rs,
                    in_=bass.AP(tensor=dr.tensor, offset=dr.offset, ap=[[16, HD], [1, 16]]),
                )
                rr = pN.tile([HD, 16], F32, tag="rr", name=f"rr_{half}_{h}")
                nc.vector.reciprocal(rr, rs)
                dr2 = pD.tile([HD, 16], F32, tag="dr2", name=f"dr2_{half}_{h}")
                nc.sync.dma_start(out=dr2, in_=rr)
                rec = pN.tile([HD, HF], F32, tag="rec", name=f"rec_{half}_{h}")
                nc.sync.dma_start(
                    out=rec,
                    in_=bass.AP(tensor=dr2.tensor, offset=dr2.offset, ap=[[0, HD], [1, HF]]),
                )
                return un, rec

            def emit_norm_finish(half, h, un, rec):
                hs = slice(half * HF, (half + 1) * HF)
                nc.vector.tensor_mul(
                    oT_sb[h // 2][(h % 2) * HD : (h % 2) * HD + HD, hs],
                    un[0:HD, :],
                    rec,
                )

            # Fillers: C mt j must be emitted by unit-0 slot j (PV(0,j)
            # consumes v_sb[j]); B m1/m3 must complete before unit 2
            # (heads 2-3) -> unit-1 slots 0-7.
            fillers = [filler_c(mt) for mt in range(NTT)]
            fillers += [filler_b(1, ch) for ch in range(4)]
            fillers += [filler_b(3, ch) for ch in range(4)]

            units = [(half, h) for half in range(2) for h in range(HPC)]
            prev = None
            for i, (half, h) in enumerate(units):
                po = emit_unit(half, h, fillers)
                nc_unit = (half, h, po)
                cur = emit_norm_copy(half, h, po)
                if prev is not None:
                    emit_norm_finish(prev[0], prev[1], prev[2], prev[3])
                    if i >= 4:
                        emit_outproj([2 * (i - 4), 2 * (i - 4) + 1], act_copies=False)
                prev = (half, h, cur[0], cur[1])
            emit_norm_finish(prev[0], prev[1], prev[2], prev[3])
            emit_outproj(list(range(8, 16)), act_copies=True)

    # bass.Bass's finalize skips Bacc's wait-splitting passes; walrus allows
    # at most 1 sync wait per instruction (2 for event semaphores), so run
    # just those two passes here.
    import bass_rust as _bass_rust

    _bass_rust.move_matmul_waits_to_ldweights(nc.m)
    _bass_rust.generate_event_semaphores(nc)
    return nc


def prepare_in_maps(inputs):
    q = np.asarray(inputs["query"], np.float32)
    ipw = np.asarray(inputs["in_proj_weight"], np.float32)
    ipb = np.asarray(inputs["in_proj_bias"], np.float32)
    out_w = np.asarray(inputs["out_w"], np.float32)
    k_a = np.asarray(inputs["k_a"], np.float32)
    k_b = np.asarray(inputs["k_b"], np.float32)
    v_a = np.asarray(inputs["v_a"], np.float32)
    v_b = np.asarray(inputs["v_b"], np.float32)
    qscale = 1.0 / math.sqrt(HD)
    sl = SCALE / R

    in_maps = []
    for c in range(NCORES):
        bb = c // 4
        s = (c % 4) * CD
        e = s + CD
        X = q[:, bb, :]

        xa = X.T  # (D, T)

        wqk = np.zeros((D, 2 * CD), np.float32)
        wqk[:, :CD] = ipw[s:e].T * qscale
        wqk[:, CD:] = ipw[D + s : D + e].T

        wv = np.zeros((D, HPC * VW), np.float32)
        for j in range(HPC):
            wv[:, j * VW : j * VW + HD] = ipw[2 * D + s + j * HD : 2 * D + s + (j + 1) * HD].T

        ab = np.zeros((D, 3 * R), np.float32)
        ab[:, :R] = k_a.T
        ab[:, 2 * R :] = v_a.T

        kbm = np.zeros((RA, CD), np.float32)
        kbm[:R] = k_b[:, s:e] * sl
        kbm[R] = ipb[D + s : D + e]  # K bias via ones row

        vbm = np.zeros((RA, HPC * VW), np.float32)
        for j in range(HPC):
            vbm[:R, j * VW : j * VW + HD] = v_b[:, s + j * HD : s + (j + 1) * HD] * sl
            vbm[R, j * VW : j * VW + HD] = ipb[2 * D + s + j * HD : 2 * D + s + (j + 1) * HD]
            vbm[R, j * VW + HD] = 1.0  # denominator ones column

        qbias = np.stack([ipb[s : s + P], ipb[s + P : s + 2 * P]], axis=1) * qscale

        wo = out_w[:, s:e].T

        in_maps.append(
            {
                "xa": xa.astype(BF16),
                "wqk": wqk.astype(BF16),
                "wv": wv.astype(BF16),
                "ab": ab.astype(BF16),
                "kbm": kbm.astype(BF16),
                "vbm": vbm.astype(BF16),
                "qb": qbias.astype(np.float32),
                "wo": wo.astype(BF16),
            }
        )
    return in_maps


def assemble_output(inputs, results):
    out_b = np.asarray(inputs["out_b"], np.float32)
    out = np.zeros((T, BSZ, D), np.float32)
    for c in range(NCORES):
        out[:, c // 4, :] += results[c]["out"].astype(np.float32)
    out += out_b[None, None, :]
    return out


def kernel(**inputs):
    nc = build_nc()
    in_maps = prepare_in_maps(inputs)
    res = run_bass_kernel_spmd(nc, in_maps, core_ids=list(range(NCORES)))
    return assemble_output(inputs, res.results)
